# revision 120
# baseline (speedup 1.0000x reference)
"""GroupMultiHeadAttention (GQA, causal, RoPE) Trainium2 Bass kernel.

Problem: x[1,2048,2048] -> MHA with H=32 heads, G=8 KV groups (4 heads/group),
head_dim=64, causal mask, RoPE on q/k, out proj. f32.

Sharding: 8-way tensor parallel by heads. Core c owns heads 4c..4c+3
(= KV group c): Wq/Wk/Wv column-sharded, Wo row-sharded. Each core produces
a partial y^T [D, L]; the host sums the 8 partials and transposes (this is
the gather/unshard step; no on-device collective needed).

Device-side strategy (per core):
  - xT [d, l] is prepared on the host (layout transform, same bytes as x,
    bf16) and streamed per 512-column l-block; no on-chip transposes.
    Weights are host-pre-tiled to [128, k*f] so every DMA moves >=512B
    contiguous elements with minimal descriptor counts.
  - Projections compute qT/kT/vT ([feat, l], f32 PSUM accumulation over 16
    k-tiles). Wk columns are host-duplicated (wktd) so kT lands duplicated
    in both partition halves for free - two heads per scores matmul.
  - RoPE: rotate_half is a 128x128 constant permutation matmul (PT), then
    q = raw*cosT + rot*sinT on DVE; each PT matmul is emitted behind the
    NEXT projection chain so its raw-copy latency hides under matmuls.
  - Scores sT[kl, ql] in f32r; the causal mask is folded into the PSUM
    accumulation as an additive -1e12 matmul (identity lhsT x mask tile)
    covering only the 128-wide diagonal boundary; fully-masked column
    prefixes of diagonal strips are skipped in scores/exp/av entirely.
  - Softmax without max-subtraction: exp on ACT straight out of PSUM
    (scale=1/8 fused) into bf16 probs; denominator comes free from a ones
    column appended to v (v_aug [kl, 65]); normalization broadcasts the
    reciprocal row via a PE rank-1 outer product (ones x recip) and one
    DVE multiply; the second head of each pair is shifted to partitions
    64..127 by a small gpsimd SBUF-SBUF DMA.
  - Out-projections are deferred and emitted after all attention blocks so
    projections/attention of later blocks fill the ACT-bound windows; yp
    tiles alternate between two PSUM pools and yT stores go out as paired
    256-row bf16 DMAs.
  - Emission order is software-pipelined: proj(0); for j: attn(j),
    proj(j+1); all outproj(j) at the end. PSUM: ps_s 2 banks (proj accs +
    even yp), ps_b 4 banks (scores pairs, rope/broadcast rank-1s, odd yp),
    ps_o 2 banks (oA/oB accumulators).
"""

import os
import ml_dtypes
import numpy as np

import concourse.bass as bass
import concourse.tile as tile
from concourse import mybir
from concourse.bass_utils import run_bass_kernel_spmd

F32R = mybir.dt.float32r
F32 = mybir.dt.float32
BF16 = mybir.dt.bfloat16

L = 2048          # sequence length
D = 2048          # model dim
HD = 64           # head dim
NHC = 4           # heads per core
FEAT = NHC * HD   # 256 per-core q features
LB = 512          # l block size
NLB = L // LB     # 4
KT = D // 128     # 16 contraction tiles
NCORES = 8


def _build_bass():
    nc = bass.Bass()

    xt = nc.dram_tensor("xt", [D, L], BF16, kind="ExternalInput")
    wqt = nc.dram_tensor("wqt", [128, KT * FEAT], BF16, kind="ExternalInput")
    wktd = nc.dram_tensor("wktd", [128, KT * 128], BF16, kind="ExternalInput")
    wvt = nc.dram_tensor("wvt", [128, KT * HD], BF16, kind="ExternalInput")
    wot = nc.dram_tensor("wot", [FEAT, D], BF16, kind="ExternalInput")
    cost2 = nc.dram_tensor("cost2", [128, L], BF16, kind="ExternalInput")
    sint2 = nc.dram_tensor("sint2", [128, L], BF16, kind="ExternalInput")
    ptm = nc.dram_tensor("ptm", [128, 128], F32R, kind="ExternalInput")
    ltri = nc.dram_tensor("ltri", [128, 4 * LB], BF16, kind="ExternalInput")
    onesc = nc.dram_tensor("onesc", [128, KT], BF16, kind="ExternalInput")
    onesr = nc.dram_tensor("onesr", [128, HD], F32R, kind="ExternalInput")
    idenh = nc.dram_tensor("idenh", [128, 128], BF16, kind="ExternalInput")
    yt = nc.dram_tensor("yt", [D, L], BF16, kind="ExternalOutput")

    with tile.TileContext(nc) as tc:
        with (
            tc.tile_pool(name="singles", bufs=1) as singles,
            tc.tile_pool(name="xt", bufs=6) as xt_p,
            tc.tile_pool(name="rtmp", bufs=3) as rtmp_p,
            tc.tile_pool(name="probs", bufs=4) as probs_p,
            tc.tile_pool(name="otmp", bufs=2) as otmp_p,
            tc.tile_pool(name="osum", bufs=2) as osum_p,
            tc.tile_pool(name="obc", bufs=3) as obc_p,
            tc.tile_pool(name="outsb", bufs=4) as outsb_p,
            tc.tile_pool(name="vtt", bufs=2) as vtt_p,
            tc.tile_pool(name="ytsb", bufs=8) as ytsb_p,
            tc.tile_pool(name="ps_s", bufs=2, space="PSUM") as ps_s,
            tc.tile_pool(name="ps_o", bufs=2, space="PSUM") as ps_o,
            tc.tile_pool(name="ps_b", bufs=2, space="PSUM") as ps_b,
        ):
            # ---- resident tensors --------------------------------------
            # weights/consts dispatch on the Pool SWDGE queue so the SP HW
            # queue is free for the x stream; wktd (k weights duplicated on
            # the host into both column halves) first so the k chain starts
            # earliest.
            wkt_sb = singles.tile([128, KT, 128], BF16)
            # first k-tile group via the idle ACT HWDGE queue so the first
            # chain starts ~2us earlier; the rest streams on the Pool queue
            nc.scalar.dma_start(
                wkt_sb[:, 0:4, :],
                wktd[:, 0:4 * 128].rearrange("p (k f) -> p k f", k=4))
            nc.gpsimd.dma_start(
                wkt_sb[:, 4:KT, :],
                wktd[:, 4 * 128:].rearrange("p (k f) -> p k f", k=KT - 4))
            pt_sb = singles.tile([128, 128], F32R)
            nc.gpsimd.dma_start(pt_sb, ptm[:, :])
            wqt_sb = singles.tile([128, KT, FEAT], BF16)
            nc.scalar.dma_start(
                wqt_sb[:, 0:4, :],
                wqt[:, 0:4 * FEAT].rearrange("p (k f) -> p k f", k=4))
            nc.gpsimd.dma_start(
                wqt_sb[:, 4:KT, :],
                wqt[:, 4 * FEAT:].rearrange("p (k f) -> p k f", k=KT - 4))
            wvt_sb = singles.tile([128, KT, HD], BF16)
            nc.scalar.dma_start(
                wvt_sb, wvt.rearrange("p (k f) -> p k f", k=KT))
            idh_sb = singles.tile([128, 128], BF16)
            nc.gpsimd.dma_start(idh_sb, idenh[:, :])
            cos_sb = singles.tile([128, L], BF16)
            nc.scalar.dma_start(cos_sb, cost2[:, :])
            sin_sb = singles.tile([128, L], BF16)
            nc.gpsimd.dma_start(sin_sb, sint2[:, :])
            ltri_sb = singles.tile([128, 4, LB], BF16)
            nc.gpsimd.dma_start(
                ltri_sb, ltri.rearrange("p (t q) -> p t q", t=4))
            ones_sb = singles.tile([128, HD], F32R)
            nc.gpsimd.dma_start(ones_sb, onesr[:, :])
            wot_sb = singles.tile([128, 2, D], BF16)
            nc.gpsimd.dma_start(wot_sb, wot.rearrange("(t p) d -> p t d",
                                                      p=128))
            weights_loaded = [False]

            qt_sb = singles.tile([128, 2, L], F32R)     # roped qT, head pairs
            ktd_sb = singles.tile([128, L], F32R)       # roped kT, duplicated
            vaug_sb = singles.tile([128, KT, HD + 1], BF16)  # v with ones col
            nc.gpsimd.dma_start(vaug_sb[:, :, HD:HD + 1],
                                onesc.rearrange("p (k o) -> p k o", o=1))

            copy_flip = [0]

            def copy_out(dst, src):
                # alternate PSUM->SBUF copies between DVE and ACT
                if copy_flip[0] % 2 == 0:
                    nc.vector.tensor_copy(dst, src)
                else:
                    nc.scalar.copy(dst, src)
                copy_flip[0] += 1

            def copy_split(dst, src):
                # latency-critical copy: DVE and ACT each move half the rows
                nc.vector.tensor_copy(dst[0:64], src[0:64])
                nc.scalar.copy(dst[64:128], src[64:128])

            def emit_proj(j):
                """qT/kT/vT projections + rope for l-block j."""
                jsl = bass.ts(j, LB)
                # ---- load xT columns for this l-block (4 chunks) --------
                xt_c = []
                for c in range(4):
                    xc = xt_p.tile([128, 4, LB], BF16, tag="xt")
                    nc.sync.dma_start(
                        xc,
                        xt[c * 512:(c + 1) * 512, jsl].rearrange(
                            "(k p) l -> p k l", p=128),
                    )
                    xt_c.append(xc)

                def accumulate(lhs_of_k, m):
                    acc = ps_s.tile([128, LB], F32, tag="ps_s")
                    for k in range(KT):
                        nc.tensor.matmul(
                            acc[:m, :], lhs_of_k(k),
                            xt_c[k // 4][:, k % 4, :],
                            start=(k == 0), stop=(k == KT - 1),
                        )
                    return acc

                def rope_into(dst, raw, rps):
                    # dst = raw * cos + rot(raw) * sin   (for this l block)
                    tmp = rtmp_p.tile([128, LB], F32R, tag="ropetmp")
                    nc.vector.tensor_mul(tmp, rps, sin_sb[:, jsl])
                    nc.vector.tensor_mul(dst, raw, cos_sb[:, jsl])
                    nc.vector.tensor_add(dst, dst, tmp)

                # chain order k, q0, q1, v with each PT-rope emitted
                # behind the NEXT chain, so the raw-copy latency hides
                # under that chain's matmuls instead of stalling PE.
                acc = accumulate(lambda k: wkt_sb[:, k, :], 128)
                kraw = rtmp_p.tile([128, LB], F32R, tag="raw")
                copy_out(kraw, acc)

                acc = accumulate(lambda k: wqt_sb[:, k, 0:128], 128)
                raw0 = rtmp_p.tile([128, LB], F32R, tag="raw")
                copy_out(raw0, acc)

                rpsw = ps_b.tile([128, 2, LB], F32, tag="ps_b")
                rps = rpsw[:, 0, :]
                nc.tensor.matmul(rps, pt_sb, kraw, start=True, stop=True)
                rope_into(ktd_sb[:, jsl], kraw, rps)

                acc = accumulate(lambda k: wqt_sb[:, k, 128:256], 128)
                raw1 = rtmp_p.tile([128, LB], F32R, tag="raw")
                copy_out(raw1, acc)

                rpsw = ps_b.tile([128, 2, LB], F32, tag="ps_b")
                rps = rpsw[:, 0, :]
                nc.tensor.matmul(rps, pt_sb, raw0, start=True, stop=True)
                rope_into(qt_sb[:, 0, jsl], raw0, rps)

                acc = accumulate(lambda k: wvt_sb[:, k, :], HD)
                vt_t = vtt_p.tile([HD, LB], BF16, tag="vtt")
                nc.vector.tensor_copy(vt_t, acc[0:HD, :])

                rpsw = ps_b.tile([128, 2, LB], F32, tag="ps_b")
                rps = rpsw[:, 0, :]
                nc.tensor.matmul(rps, pt_sb, raw1, start=True, stop=True)
                rope_into(qt_sb[:, 1, jsl], raw1, rps)

                tp = ps_s.tile([128, LB], BF16, tag="ps_s")
                for i in range(4):
                    nc.tensor.transpose(
                        tp[:, i * HD:(i + 1) * HD],
                        vt_t[:, i * 128:(i + 1) * 128],
                        idh_sb[0:HD, 0:HD],
                    )
                for i in range(4):
                    copy_out(vaug_sb[:, 4 * j + i, 0:HD],
                             tp[:, i * HD:(i + 1) * HD])

            def emit_attn(j):
                """causal attention for ql block j -> normalized out_t."""
                jsl = bass.ts(j, LB)
                nkl = 4 * (j + 1)          # causal kl tiles
                out_t = outsb_p.tile([128, 2, LB], BF16, tag="outsb")
                for fb in range(2):
                    oA = ps_o.tile([HD + 1, LB], F32, tag="ps_o")
                    oB = ps_o.tile([HD + 1, LB], F32, tag="ps_o")
                    # off-diagonal kl tiles: full width, no mask
                    for pi in range(2 * j):
                        t0 = 2 * pi
                        sA = ps_b.tile([128, 2, LB], F32, tag="ps_b")
                        sB = ps_b.tile([128, 2, LB], F32, tag="ps_b")
                        for ti in range(2):
                            t = t0 + ti
                            ksl = bass.ts(t, 128)
                            nc.tensor.matmul(
                                sA[:, ti, :], ktd_sb[0:HD, ksl],
                                qt_sb[0:HD, fb, jsl],
                                start=True, stop=True)
                            nc.tensor.matmul(
                                sB[:, ti, :], ktd_sb[HD:128, ksl],
                                qt_sb[HD:128, fb, jsl],
                                start=True, stop=True)
                        pA = probs_p.tile([128, 2, LB], BF16, tag="probs")
                        pB = probs_p.tile([128, 2, LB], BF16, tag="probs")
                        nc.scalar.activation(
                            pA, sA, mybir.ActivationFunctionType.Exp,
                            scale=0.125)
                        nc.scalar.activation(
                            pB, sB, mybir.ActivationFunctionType.Exp,
                            scale=0.125)
                        for ti in range(2):
                            t = t0 + ti
                            nc.tensor.matmul(
                                oA, vaug_sb[:, t, :], pA[:, ti, :],
                                start=(t == 0), stop=False)
                            nc.tensor.matmul(
                                oB, vaug_sb[:, t, :], pB[:, ti, :],
                                start=(t == 0), stop=False)
                    # diagonal strips in two groups of 2; columns left of
                    # the group start are fully masked and skipped through
                    # scores/exp/av; the additive -1e12 mask only needs the
                    # 128-wide triangular boundary of each strip.
                    for g in range(2):
                        cg = 256 * g
                        qsl = slice(j * LB + cg, (j + 1) * LB)
                        sA = ps_b.tile([128, 2, LB], F32, tag="ps_b")
                        sB = ps_b.tile([128, 2, LB], F32, tag="ps_b")
                        for ti in range(2):
                            i = 2 * g + ti
                            t = 4 * j + i
                            ksl = bass.ts(t, 128)
                            nc.tensor.matmul(
                                sA[:, ti, cg:], ktd_sb[0:HD, ksl],
                                qt_sb[0:HD, fb, qsl],
                                start=True, stop=False)
                            nc.tensor.matmul(
                                sB[:, ti, cg:], ktd_sb[HD:128, ksl],
                                qt_sb[HD:128, fb, qsl],
                                start=True, stop=False)
                        # maskadds grouped so the shared identity lhsT is
                        # loaded once (legalizer dedups adjacent ldweights)
                        for ti in range(2):
                            i = 2 * g + ti
                            ci = 128 * i
                            nc.tensor.matmul(
                                sA[:, ti, ci:ci + 128], idh_sb,
                                ltri_sb[:, i, ci:ci + 128],
                                start=False, stop=True)
                            nc.tensor.matmul(
                                sB[:, ti, ci:ci + 128], idh_sb,
                                ltri_sb[:, i, ci:ci + 128],
                                start=False, stop=True)
                        pA = probs_p.tile([128, 2, LB], BF16, tag="probs")
                        pB = probs_p.tile([128, 2, LB], BF16, tag="probs")
                        nc.scalar.activation(
                            pA[:, :, cg:], sA[:, :, cg:],
                            mybir.ActivationFunctionType.Exp,
                            scale=0.125)
                        nc.scalar.activation(
                            pB[:, :, cg:], sB[:, :, cg:],
                            mybir.ActivationFunctionType.Exp,
                            scale=0.125)
                        for ti in range(2):
                            i = 2 * g + ti
                            t = 4 * j + i
                            ci = 128 * i
                            last = t == nkl - 1
                            nc.tensor.matmul(
                                oA[:, ci:], vaug_sb[:, t, :],
                                pA[:, ti, ci:],
                                start=(t == 0), stop=last)
                            nc.tensor.matmul(
                                oB[:, ci:], vaug_sb[:, t, :],
                                pB[:, ti, ci:],
                                start=(t == 0), stop=last)
                    # normalize: divide by the ones-row sums. The reciprocal
                    # row (partition 64) is broadcast to partitions 0..64 by
                    # a PE rank-1 outer product with a ones column.
                    for half, oX in ((0, oA), (1, oB)):
                        sums = osum_p.tile([HD + 1, LB], F32R, tag="osum")
                        with nc.allow_low_precision(reason="f32r is f32"):
                            nc.vector.reciprocal(sums[HD:HD + 1, :],
                                                 oX[HD:HD + 1, :])
                        bcpw = ps_b.tile([128, 2, LB], F32, tag="ps_b")
                        bcp = bcpw[:, 0, :]
                        nc.tensor.matmul(bcp[0:HD, :],
                                         ones_sb[HD:HD + 1, 0:HD],
                                         sums[HD:HD + 1, :],
                                         start=True, stop=True)
                        bcs = obc_p.tile([HD, LB], F32R, tag="obc")
                        copy_out(bcs, bcp[0:HD, :])
                        if half == 0:
                            nc.vector.tensor_mul(
                                out_t[0:HD, fb, :], oX[0:HD, :], bcs)
                        else:
                            ot = otmp_p.tile([HD, LB], BF16, tag="otmp")
                            nc.vector.tensor_mul(ot, oX[0:HD, :], bcs)
                            nc.gpsimd.dma_start(out_t[HD:128, fb, :], ot)
                return out_t

            def emit_outproj(j, out_t, only_ps_s=False):
                jsl = bass.ts(j, LB)
                last = j == NLB - 1
                for dp in range(KT // 2):
                    ys = ytsb_p.tile([128, 2, LB], BF16, tag="ytsb")
                    for u in range(2):
                        dt = 2 * dp + u
                        # on the last block ps_b is idle: use it for every
                        # other yp to deepen the out-proj pipeline
                        if dt % 2 == 1 and not only_ps_s:
                            ypw = ps_b.tile([128, 2, LB], F32, tag="ps_b")
                            yp = ypw[:, 0, :]
                        else:
                            yp = ps_s.tile([128, LB], F32, tag="ps_s")
                        for kf in range(2):
                            nc.tensor.matmul(
                                yp, wot_sb[:, kf, dt * 128:(dt + 1) * 128],
                                out_t[:, kf, :],
                                start=(kf == 0), stop=(kf == 1))
                        copy_out(ys[:, u, :], yp)
                    nc.sync.dma_start(
                        yt[dp * 256:(dp + 1) * 256, jsl].rearrange(
                            "(u p) l -> p u l", p=128),
                        ys)

            # software pipeline: proj(j+1) is emitted before outproj(j) so
            # the shared ps_s rotation lets projections fill the ACT-bound
            # attention window instead of serializing behind out-proj.
            emit_proj(0)
            pend = []
            for j in range(NLB):
                out_t = emit_attn(j)
                if j + 1 < NLB:
                    emit_proj(j + 1)
                pend.append((j, out_t))
                if j == NLB - 2:
                    jo, ot_ = pend.pop(0)
                    emit_outproj(jo, ot_, only_ps_s=True)
                    jo, ot_ = pend.pop(0)
                    emit_outproj(jo, ot_, only_ps_s=True)
            for jo, ot_ in pend:
                emit_outproj(jo, ot_)

    return nc


def _split_waits(nc, keep=1):
    """walrus in this container encodes at most one sync-wait per
    instruction; hoist extra waits into preceding same-engine NoOps."""
    for fn in nc.m.functions:
        for blk in fn.blocks:
            newl = []
            for ins in blk.instructions:
                si = ins.sync_info
                if (si is not None and si.on_wait is not None
                        and len(si.on_wait) > keep):
                    waits = list(si.on_wait)
                    extra, last = waits[:-keep], waits[-keep:]
                    for i, w in enumerate(extra):
                        nop = mybir.InstNoOp(name=f"{ins.name}-w{i}")
                        nop.engine = ins.engine
                        nop.sync_info = mybir.SyncInfo(on_wait=[w],
                                                       on_update=[])
                        newl.append(nop)
                    si.on_wait = last
                    ins.sync_info = si
                newl.append(ins)
            blk.instructions = newl


_NC_CACHE = None


def _get_nc():
    global _NC_CACHE
    if _NC_CACHE is None:
        _NC_CACHE = _build_bass()
        _split_waits(_NC_CACHE)
    return _NC_CACHE


def _host_prep(x, mask, cos, sin, Wq, Wk, Wv, Wo):
    """Build the 8 per-core input maps (sharding + layout transforms)."""
    x2d = np.ascontiguousarray(x.reshape(L, D).astype(np.float32))
    xtp = np.ascontiguousarray(x2d.T.astype(ml_dtypes.bfloat16))  # [D, L]

    cosT = np.ascontiguousarray(cos.T.astype(ml_dtypes.bfloat16))  # [64, L]
    sinT = np.ascontiguousarray(sin.T.astype(ml_dtypes.bfloat16))
    cost2 = np.concatenate([cosT, cosT], axis=0)              # [128, L]
    sint2 = np.concatenate([sinT, sinT], axis=0)

    # rotate_half as a left-multiplication in [hd, l] layout:
    # rot(v) = P @ v with P[d, d+32] = -1 (d<32), P[d, d-32] = 1 (d>=32)
    P = np.zeros((HD, HD), dtype=np.float32)
    P[np.arange(32), np.arange(32) + 32] = -1.0
    P[np.arange(32, 64), np.arange(32, 64) - 32] = 1.0
    PT = P.T  # lhsT for the matmul
    ptm = np.zeros((128, 128), dtype=np.float32)
    ptm[0:64, 0:64] = PT
    ptm[64:128, 64:128] = PT
    ptm = np.ascontiguousarray(ptm)

    # additive causal mask (0 keep / -1e12 masked), transposed, diag block
    keepT = np.logical_not(np.asarray(mask)).T
    ltri_f = np.where(keepT[:LB, :LB], 0.0, -1e12)
    ltri = np.ascontiguousarray(
        ltri_f.reshape(4, 128, LB).transpose(1, 0, 2).reshape(
            128, 4 * LB).astype(ml_dtypes.bfloat16))

    in_maps = []
    for c in range(NCORES):
        fs = slice(c * FEAT, (c + 1) * FEAT)
        gs = slice(c * HD, (c + 1) * HD)
        wk_t = Wk[gs, :].T.astype(np.float32)         # [D, 64]

        def pretile(w):
            # [D, F] -> [128, KT*F]: partition p holds k-tile rows p+128k
            dd, ff = w.shape
            return np.ascontiguousarray(
                w.reshape(KT, 128, ff).transpose(1, 0, 2).reshape(
                    128, KT * ff).astype(ml_dtypes.bfloat16))
        in_maps.append({
            "xt": xtp,
            "wqt": pretile(Wq[fs, :].T),
            "wktd": pretile(np.concatenate([wk_t, wk_t], axis=1)),
            "wvt": pretile(Wv[gs, :].T),
            "wot": np.ascontiguousarray(Wo[:, fs].T.astype(ml_dtypes.bfloat16)),
            "cost2": cost2,
            "sint2": sint2,
            "ptm": ptm,
            "ltri": ltri,
            "onesc": np.ones((128, KT), dtype=ml_dtypes.bfloat16),
            "onesr": np.ones((128, HD), dtype=np.float32),
            "idenh": np.eye(128, dtype=ml_dtypes.bfloat16),
        })
    return in_maps


def _combine(results):
    acc = results[0]["yt"].astype(np.float32)
    for r in results[1:]:
        acc = acc + r["yt"]
    return np.ascontiguousarray(acc.T)[None, :, :].astype(np.float32)


def kernel(**inputs):
    nc = _get_nc()
    in_maps = _host_prep(**inputs)
    res = run_bass_kernel_spmd(nc, in_maps, list(range(NCORES)))
    return _combine(res.results)


def kernel_profiled(**inputs):
    """Like kernel() but returns (output, exec_time_ns, raw BassKernelResults)."""
    nc = _get_nc()
    in_maps = _host_prep(**inputs)
    res = run_bass_kernel_spmd(nc, in_maps, list(range(NCORES)), trace=True)
    return _combine(res.results), res.exec_time_ns, res
